# revision 10
# baseline (speedup 1.0000x reference)
"""Trainium2 Bass kernel for nn_AdaptiveEmbeddingI2T (8 NeuronCores).

Math (algebraically collapsed from the reference):
  img_repr r_i = mean_R img[i];  gamma/beta = MLP(r_i)
  pm_j = masked-mean_t cap[j]  (weights 1/len, BN folded out)
  BN stats: mean,var over all (B,T) per feature
  With gi = (1+gamma)*invstd, diff = beta - gi*mean:
    txt_ij = gi*pm_j + diff   (per feature)
    num    = (r o gi) . pm_j + r . diff            = P1.pm + t_i
    ||txt||^2 = gi^2 . pm^2 + 2(gi o diff) . pm + ||diff||^2
                                                   = P2.pm2 + P3x2.pm + s_i
    sim[i,j] = invn_i * num / (sqrt(||txt||^2) + 1e-8),  invn = 1/(||r||+1e-8)
  Output is sim.T  (caption-major).

Sharding: images and captions both split 8 ways (32 each per core). One
AllGather exchanges each core's (128, 272) block = 8 d-chunks x
[32 pm columns | sum(x) | sum(x^2)] in transposed (d-on-partition) layout.
"""

import os
import sys

sys.path.insert(0, "/opt/trn_rl_repo")

import numpy as np
import ml_dtypes

BF16_NP = ml_dtypes.bfloat16

from concourse import bacc, bass, mybir, tile
from concourse.alu_op_type import AluOpType
from concourse.bass_utils import run_bass_kernel_spmd

NCORES = 8
B, T, R, D, H = 256, 72, 36, 1024, 128
BL = B // NCORES            # 32 images / captions per core
CAP_ROWS = BL * T           # 2304
IMG_ROWS = BL * R           # 1152
NT_CAP = CAP_ROWS // 128    # 18
NT_IMG = IMG_ROWS // 128    # 9
NCH = D // 128              # 8 feature chunks
NBT = float(B * T)          # BN sample count
EPS_BN = 1e-5
EPS_L2 = 1e-8

F32 = mybir.dt.float32
BF16 = mybir.dt.bfloat16
Act = mybir.ActivationFunctionType


def _build_kernel():
    nc = bacc.Bacc(None, num_devices=NCORES)

    p = {}

    def param(name, shape, dt=F32):
        p[name] = nc.declare_dram_parameter(name, list(shape), dt, isOutput=False)
        return p[name]

    cap = param("cap", (CAP_ROWS, D))
    img = param("img", (IMG_ROWS, D))
    wsel = param("wsel", (CAP_ROWS, 33), BF16)   # masked-mean weights + ones col
    simg = param("simg", (IMG_ROWS, BL), BF16)   # region-mean selection (1/36)
    wg1 = param("wg1", (D, H), BF16)
    wb1 = param("wb1", (D, H), BF16)
    wg2 = param("wg2", (H, D), BF16)
    wb2 = param("wb2", (H, D), BF16)
    bg1 = param("bg1", (H, 1))
    bb1 = param("bb1", (H, 1))
    bg2p1 = param("bg2p1", (D, 1))         # bg2 + 1  (so gT = 1+gamma directly)
    bb2 = param("bb2", (D, 1))
    ident = param("ident", (128, 128))
    out = nc.declare_dram_parameter("out", [BL, B], F32, isOutput=True)
    dbg = None
    if os.environ.get("KERNEL_DEBUG"):
        dbg = {
            "dbg_gsrc": nc.declare_dram_parameter("dbg_gsrc", [128, NCH * 34], F32, isOutput=True),
            "dbg_stat": nc.declare_dram_parameter("dbg_stat", [128, 16], F32, isOutput=True),
            "dbg_pool": nc.declare_dram_parameter("dbg_pool", [128, NCH * B], F32, isOutput=True),
            "dbg_rsb": nc.declare_dram_parameter("dbg_rsb", [BL, D], F32, isOutput=True),
            "dbg_den": nc.declare_dram_parameter("dbg_den", [BL, B], F32, isOutput=True),
            "dbg_num": nc.declare_dram_parameter("dbg_num", [BL, B], F32, isOutput=True),
            "dbg_scal": nc.declare_dram_parameter("dbg_scal", [BL, 4], F32, isOutput=True),
        }

    with tile.TileContext(nc) as tc:
        _body(nc, tc, p, out, dbg)

    nc.compile()
    return nc


def _body(nc, tc, p, out, dbg=None):
    rg = [list(range(NCORES))]

    with (
        tc.tile_pool(name="capio", bufs=3) as capio,
        tc.tile_pool(name="sqp", bufs=3) as sqp,
        tc.tile_pool(name="wselp", bufs=3) as wselp,
        tc.tile_pool(name="imgio", bufs=3) as imgio,
        tc.tile_pool(name="persist", bufs=1) as pers,
        tc.tile_pool(name="coeff", bufs=1) as coeff,
        tc.tile_pool(name="pool_big", bufs=1) as poolbig,
        tc.tile_pool(name="dram", bufs=1, space="DRAM") as dram,
    ):
        # identity for PE transposes
        ident_sb = pers.tile([128, 128], F32)
        nc.sync.dma_start(ident_sb[:, :], p["ident"][:, :])

        pmsb = pers.tile([33, D], F32)
        s2row = pers.tile([1, D], F32)
        gsrc = pers.tile([128, NCH * 34], F32)
        rsb = pers.tile([BL, D], F32)
        rT = pers.tile([128, NCH * BL], BF16)

        with (
            tc.tile_pool(name="ps_cap", bufs=1, space="PSUM") as ps_cap,
            tc.tile_pool(name="ps_img", bufs=1, space="PSUM") as ps_img,
            tc.tile_pool(name="ps_tr", bufs=2, space="PSUM") as ps_tr,
        ):
            # ===== Phase A: captions -> pm (masked mean) + BN partials =====
            # psum_P rows 0..31 = pm, row 32 = sum(x); psum_S row 32 = sum(x^2)
            psum_P0 = ps_cap.tile([33, 512], F32, tag="p0")
            psum_P1 = ps_cap.tile([33, 512], F32, tag="p1")
            psum_S0 = ps_cap.tile([33, 512], F32, tag="s0")
            psum_S1 = ps_cap.tile([33, 512], F32, tag="s1")
            for t in range(NT_CAP):
                cap_t = capio.tile([128, D], F32, tag="cap")
                nc.sync.dma_start(cap_t[:, :], p["cap"][t * 128:(t + 1) * 128, :])
                capb = capio.tile([128, D], BF16, tag="capb")
                nc.vector.tensor_copy(capb[:, :], cap_t[:, :])
                ws_t = wselp.tile([128, 33], BF16, tag="ws")
                nc.sync.dma_start(ws_t[:, :], p["wsel"][t * 128:(t + 1) * 128, :])
                sq_t = sqp.tile([128, D], BF16, tag="sq")
                nc.scalar.activation(sq_t[:, :], cap_t[:, :], Act.Square)
                st, sp = (t == 0), (t == NT_CAP - 1)
                nc.tensor.matmul(psum_P0[:, :], ws_t[:, :], capb[:, 0:512],
                                 start=st, stop=sp)
                nc.tensor.matmul(psum_P1[:, :], ws_t[:, :], capb[:, 512:1024],
                                 start=st, stop=sp)
                nc.tensor.matmul(psum_S0[:, :], ws_t[:, :], sq_t[:, 0:512],
                                 start=st, stop=sp)
                nc.tensor.matmul(psum_S1[:, :], ws_t[:, :], sq_t[:, 512:1024],
                                 start=st, stop=sp)

            # pmsb rows 0..31 = pm (b-major), row 32 = sum(x); s2row = sum(x^2)
            nc.scalar.copy(pmsb[0:33, 0:512], psum_P0[:, :])
            nc.scalar.copy(pmsb[0:33, 512:1024], psum_P1[:, :])
            nc.scalar.copy(s2row[0:1, 0:512], psum_S0[32:33, :])
            nc.scalar.copy(s2row[0:1, 512:1024], psum_S1[32:33, :])

            # transpose to d-on-partition chunks; pack pm to cols [c*32, +32),
            # sum(x) to col 256+c, sum(x^2) to col 264+c
            for c in range(NCH):
                tp = ps_tr.tile([128, 34], F32, tag="tp")
                nc.tensor.transpose(tp[:, 0:33], pmsb[:, c * 128:(c + 1) * 128],
                                    ident_sb[0:33, 0:33])
                nc.scalar.copy(gsrc[:, c * BL:(c + 1) * BL], tp[:, 0:BL])
                nc.scalar.copy(gsrc[:, NCH * BL + c:NCH * BL + c + 1],
                               tp[:, 32:33])
                tp2 = ps_tr.tile([128, 34], F32, tag="tp")
                nc.tensor.transpose(tp2[:, 0:1], s2row[:, c * 128:(c + 1) * 128],
                                    ident_sb[0:1, 0:1])
                nc.scalar.copy(gsrc[:, NCH * BL + NCH + c:NCH * BL + NCH + c + 1],
                               tp2[:, 0:1])

            if dbg:
                nc.gpsimd.dma_start(dbg["dbg_gsrc"][:, :], gsrc[:, :])

            # ===== Phase B: AllGather (pm blocks + BN partials) =====
            cc_in = dram.tile([128, NCH * 34], F32)
            cc_out = dram.tile([128 * NCORES, NCH * 34], F32, addr_space="Shared")
            nc.gpsimd.dma_start(cc_in[:, :], gsrc[:, :])
            nc.gpsimd.collective_compute(
                "AllGather", AluOpType.bypass, replica_groups=rg,
                ins=[cc_in.opt()], outs=[cc_out.opt()],
            )

            # ===== Phase C: images (independent of collective) =====
            psum_I0 = ps_img.tile([BL, 512], F32, tag="i0")
            psum_I1 = ps_img.tile([BL, 512], F32, tag="i1")
            for t in range(NT_IMG):
                img_t = imgio.tile([128, D], F32, tag="img")
                nc.sync.dma_start(img_t[:, :], p["img"][t * 128:(t + 1) * 128, :])
                imgb = imgio.tile([128, D], BF16, tag="imgb")
                nc.vector.tensor_copy(imgb[:, :], img_t[:, :])
                si_t = imgio.tile([128, BL], BF16, tag="si")
                nc.sync.dma_start(si_t[:, :], p["simg"][t * 128:(t + 1) * 128, :])
                st, sp = (t == 0), (t == NT_IMG - 1)
                nc.tensor.matmul(psum_I0[:, :], si_t[:, :], imgb[:, 0:512],
                                 start=st, stop=sp)
                nc.tensor.matmul(psum_I1[:, :], si_t[:, :], imgb[:, 512:1024],
                                 start=st, stop=sp)
            nc.scalar.copy(rsb[:, 0:512], psum_I0[:, :])
            nc.scalar.copy(rsb[:, 512:1024], psum_I1[:, :])

            # transpose img_repr to chunk-major (128, 8*32)
            for c in range(NCH):
                tp = ps_tr.tile([128, 34], F32, tag="tp")
                nc.tensor.transpose(tp[:, 0:BL], rsb[:, c * 128:(c + 1) * 128],
                                    ident_sb[0:BL, 0:BL])
                nc.scalar.copy(rT[:, c * BL:(c + 1) * BL], tp[:, 0:BL])

        # ===== weights for the MLP =====
        wg1_sb = pers.tile([128, D], BF16)  # (p, c*128+h) <- Wg1[c*128+p, h]
        nc.sync.dma_start(
            wg1_sb[:, :].rearrange("p (c h) -> p c h", c=NCH),
            p["wg1"].ap().rearrange("(c p) h -> p c h", c=NCH))
        wb1_sb = pers.tile([128, D], BF16)
        nc.sync.dma_start(
            wb1_sb[:, :].rearrange("p (c h) -> p c h", c=NCH),
            p["wb1"].ap().rearrange("(c p) h -> p c h", c=NCH))
        wg2_sb = pers.tile([128, D], BF16)  # natural (h, d)
        nc.sync.dma_start(wg2_sb[:, :], p["wg2"][:, :])
        wb2_sb = pers.tile([128, D], BF16)
        nc.sync.dma_start(wb2_sb[:, :], p["wb2"][:, :])
        bg1_sb = pers.tile([128, 1], F32)
        nc.sync.dma_start(bg1_sb[:, :], p["bg1"][:, :])
        bb1_sb = pers.tile([128, 1], F32)
        nc.sync.dma_start(bb1_sb[:, :], p["bb1"][:, :])
        bg2p1_sb = pers.tile([128, NCH], F32)
        nc.sync.dma_start(
            bg2p1_sb[:, :].rearrange("p (c u) -> p c u", c=NCH),
            p["bg2p1"].ap().rearrange("(c p) u -> p c u", c=NCH))
        bb2_sb = pers.tile([128, NCH], F32)
        nc.sync.dma_start(
            bb2_sb[:, :].rearrange("p (c u) -> p c u", c=NCH),
            p["bb2"].ap().rearrange("(c p) u -> p c u", c=NCH))

        if dbg:
            nc.gpsimd.dma_start(dbg["dbg_rsb"][:, :], rsb[:, :])
        gT = coeff.tile([128, NCH * BL], BF16)
        bT = coeff.tile([128, NCH * BL], BF16)
        with tc.tile_pool(name="ps_mlp", bufs=2, space="PSUM") as ps_mlp:
            # MLP hidden: (128h, 32)
            psum_hg = ps_mlp.tile([128, BL], F32, tag="h")
            psum_hb = ps_mlp.tile([128, BL], F32, tag="h")
            for c in range(NCH):
                st, sp = (c == 0), (c == NCH - 1)
                nc.tensor.matmul(psum_hg[:, :], wg1_sb[:, c * 128:(c + 1) * 128],
                                 rT[:, c * BL:(c + 1) * BL], start=st, stop=sp)
                nc.tensor.matmul(psum_hb[:, :], wb1_sb[:, c * 128:(c + 1) * 128],
                                 rT[:, c * BL:(c + 1) * BL], start=st, stop=sp)
            hg = pers.tile([128, BL], BF16)
            nc.scalar.activation(hg[:, :], psum_hg[:, :], Act.Relu, bias=bg1_sb[:, 0:1])
            hb = pers.tile([128, BL], BF16)
            nc.scalar.activation(hb[:, :], psum_hb[:, :], Act.Relu, bias=bb1_sb[:, 0:1])

            # gamma+1 / beta, chunk-major T layout (128, 8*32)
            for c in range(NCH):
                pg = ps_mlp.tile([128, BL], F32, tag="gb")
                nc.tensor.matmul(pg[:, :], wg2_sb[:, c * 128:(c + 1) * 128],
                                 hg[:, :], start=True, stop=True)
                nc.scalar.activation(gT[:, c * BL:(c + 1) * BL], pg[:, :], Act.Identity,
                                     bias=bg2p1_sb[:, c:c + 1])
                pb = ps_mlp.tile([128, BL], F32, tag="gb")
                nc.tensor.matmul(pb[:, :], wb2_sb[:, c * 128:(c + 1) * 128],
                                 hb[:, :], start=True, stop=True)
                nc.scalar.activation(bT[:, c * BL:(c + 1) * BL], pb[:, :], Act.Identity,
                                     bias=bb2_sb[:, c:c + 1])

        # ===== Phase D: post-collective =====
        # stats: accumulate ranks via accumulating DMA -> (128, 16)
        # cols 0:8 = sum(x) per chunk, 8:16 = sum(x^2) per chunk
        statacc = pers.tile([128, 16], F32)
        for k in range(NCORES):
            src = cc_out[k * 128:(k + 1) * 128, NCH * BL:NCH * BL + 16]
            nc.gpsimd.dma_start(
                statacc[:, :], src,
                accum_op=(AluOpType.bypass if k == 0 else AluOpType.add))

        # pooledT (128, 8*256): chunk c cols [c*256 + jglobal]
        pooledT = poolbig.tile([128, NCH * B], F32)
        pT_view = pooledT[:, :].rearrange("p (c k j) -> p c k j", c=NCH, k=NCORES)
        for k in range(NCORES):
            src = cc_out[k * 128:(k + 1) * 128, 0:NCH * BL] \
                .rearrange("p (c j) -> p c j", c=NCH)
            nc.gpsimd.dma_start(pT_view[:, :, k, :], src)

        if dbg:
            nc.gpsimd.dma_start(dbg["dbg_stat"][:, :], statacc[:, :])
            nc.gpsimd.dma_start(dbg["dbg_pool"][:, :], pooledT[:, :])

        # BN stats -> mean, invstd per chunk (128, 8)
        meanT = pers.tile([128, NCH], F32)
        nc.vector.tensor_scalar(meanT[:, :], statacc[:, 0:8], 1.0 / NBT, None,
                                AluOpType.mult)
        ex2 = pers.tile([128, NCH], F32)
        nc.vector.tensor_scalar(ex2[:, :], statacc[:, 8:16], 1.0 / NBT, None,
                                AluOpType.mult)
        msq = pers.tile([128, NCH], F32)
        nc.vector.tensor_tensor(msq[:, :], meanT[:, :], meanT[:, :], AluOpType.mult)
        var = pers.tile([128, NCH], F32)
        nc.vector.tensor_tensor(var[:, :], ex2[:, :], msq[:, :], AluOpType.subtract)
        epsbn = pers.tile([128, 1], F32)
        nc.vector.memset(epsbn[:, :], EPS_BN)
        sd = pers.tile([128, NCH], F32)
        nc.scalar.activation(sd[:, :], var[:, :], Act.Sqrt, bias=epsbn[:, 0:1])
        invT = pers.tile([128, NCH], F32)
        nc.vector.reciprocal(invT[:, :], sd[:, :])

        # image-side coefficient tiles (chunk-major (128, 8*32))
        gi = coeff.tile([128, NCH * BL], BF16)
        tmp = coeff.tile([128, NCH * BL], BF16)
        for c in range(NCH):
            sl = slice(c * BL, (c + 1) * BL)
            nc.vector.tensor_scalar(gi[:, sl], gT[:, sl], invT[:, c:c + 1], None,
                                    AluOpType.mult)
            nc.vector.tensor_scalar(tmp[:, sl], gT[:, sl], invT[:, c:c + 1],
                                    meanT[:, c:c + 1], AluOpType.mult,
                                    AluOpType.mult)
        diff = coeff.tile([128, NCH * BL], BF16)
        nc.vector.tensor_tensor(diff[:, :], bT[:, :], tmp[:, :], AluOpType.subtract)
        P1 = coeff.tile([128, NCH * BL], BF16)
        nc.vector.tensor_tensor(P1[:, :], rT[:, :], gi[:, :], AluOpType.mult)
        P2 = coeff.tile([128, NCH * BL], BF16)
        nc.vector.tensor_tensor(P2[:, :], gi[:, :], gi[:, :], AluOpType.mult)
        P3x2 = coeff.tile([128, NCH * BL], BF16)
        nc.vector.tensor_tensor(P3x2[:, :], gi[:, :], diff[:, :], AluOpType.mult)
        nc.vector.tensor_scalar(P3x2[:, :], P3x2[:, :], 2.0, None, AluOpType.mult)
        rd = coeff.tile([128, NCH * BL], BF16)
        nc.vector.tensor_tensor(rd[:, :], rT[:, :], diff[:, :], AluOpType.mult)
        d2 = coeff.tile([128, NCH * BL], BF16)
        nc.vector.tensor_tensor(d2[:, :], diff[:, :], diff[:, :], AluOpType.mult)
        r2 = coeff.tile([128, NCH * BL], BF16)
        nc.vector.tensor_tensor(r2[:, :], rT[:, :], rT[:, :], AluOpType.mult)

        # bf16 copies of pooled + pooled^2 for the PE
        pooledTb = poolbig.tile([128, NCH * B], BF16)
        nc.vector.tensor_copy(pooledTb[:, :], pooledT[:, :])
        pooled2Tb = poolbig.tile([128, NCH * B], BF16)
        nc.vector.tensor_tensor(pooled2Tb[:, :], pooledT[:, :], pooledT[:, :],
                                AluOpType.mult)

        ones_sb = pers.tile([128, 1], BF16)
        nc.vector.memset(ones_sb[:, :], 1.0)

        with tc.tile_pool(name="ps_fin", bufs=1, space="PSUM") as ps_fin:
            # per-image scalars via ones-matmuls -> (32, 1) psums
            psum_t = ps_fin.tile([BL, 1], F32, tag="sct")
            psum_s = ps_fin.tile([BL, 1], F32, tag="scs")
            psum_r2 = ps_fin.tile([BL, 1], F32, tag="scr")
            for c in range(NCH):
                st, sp = (c == 0), (c == NCH - 1)
                sl = slice(c * BL, (c + 1) * BL)
                nc.tensor.matmul(psum_t[:, :], rd[:, sl], ones_sb[:, :],
                                 start=st, stop=sp)
                nc.tensor.matmul(psum_s[:, :], d2[:, sl], ones_sb[:, :],
                                 start=st, stop=sp)
                nc.tensor.matmul(psum_r2[:, :], r2[:, sl], ones_sb[:, :],
                                 start=st, stop=sp)
            t_col = pers.tile([BL, 1], F32)
            nc.scalar.copy(t_col[:, :], psum_t[:, :])
            s_col = pers.tile([BL, 1], F32)
            nc.scalar.copy(s_col[:, :], psum_s[:, :])
            nrm = pers.tile([BL, 1], F32)
            nc.scalar.activation(nrm[:, :], psum_r2[:, :], Act.Sqrt)
            nrme = pers.tile([BL, 1], F32)
            nc.vector.tensor_scalar(nrme[:, :], nrm[:, :], EPS_L2, None, AluOpType.add)
            invn = pers.tile([BL, 1], F32)
            nc.vector.reciprocal(invn[:, :], nrme[:, :])

            # ===== Phase E: final matmuls + epilogue =====
            psum_A = ps_fin.tile([BL, B], F32, tag="A")
            psum_D = ps_fin.tile([BL, B], F32, tag="Dd")
            for c in range(NCH):
                st, sp = (c == 0), (c == NCH - 1)
                isl = slice(c * BL, (c + 1) * BL)
                jsl = slice(c * B, (c + 1) * B)
                nc.tensor.matmul(psum_A[:, :], P1[:, isl], pooledTb[:, jsl],
                                 start=st, stop=sp)
                nc.tensor.matmul(psum_D[:, :], P2[:, isl], pooled2Tb[:, jsl],
                                 start=st, stop=False)
                nc.tensor.matmul(psum_D[:, :], P3x2[:, isl], pooledTb[:, jsl],
                                 start=False, stop=sp)

            den = pers.tile([BL, B], F32)
            nc.scalar.activation(den[:, :], psum_D[:, :], Act.Sqrt, bias=s_col[:, 0:1])
            dene = pers.tile([BL, B], F32)
            nc.vector.tensor_scalar(dene[:, :], den[:, :], EPS_L2, None, AluOpType.add)
            rec = pers.tile([BL, B], F32)
            nc.vector.reciprocal(rec[:, :], dene[:, :])
            num = pers.tile([BL, B], F32)
            nc.vector.tensor_scalar(num[:, :], psum_A[:, :], t_col[:, 0:1],
                                    invn[:, 0:1], AluOpType.add, AluOpType.mult)
            if dbg:
                nc.gpsimd.dma_start(dbg["dbg_den"][:, :], den[:, :])
                nc.gpsimd.dma_start(dbg["dbg_num"][:, :], num[:, :])
                scal4 = pers.tile([BL, 4], F32)
                nc.scalar.copy(scal4[:, 0:1], t_col[:, :])
                nc.scalar.copy(scal4[:, 1:2], s_col[:, :])
                nc.scalar.copy(scal4[:, 2:3], invn[:, :])
                nc.scalar.copy(scal4[:, 3:4], nrm[:, :])
                nc.gpsimd.dma_start(dbg["dbg_scal"][:, :], scal4[:, :])
            sim_sb = pers.tile([BL, B], F32)
            nc.vector.tensor_tensor(sim_sb[:, :], num[:, :], rec[:, :], AluOpType.mult)
            nc.gpsimd.dma_start(out[:, :], sim_sb[:, :])


_NC_CACHE = None


def _get_nc():
    global _NC_CACHE
    if _NC_CACHE is None:
        _NC_CACHE = _build_kernel()
    return _NC_CACHE


def _install_ntff_shim():
    """Expose the axon NTFF profile hook so trace=True works (best effort)."""
    import types
    if "antenv.axon_hooks" in sys.modules:
        return
    try:
        mod = types.ModuleType("antenv.axon_hooks")
        state = {"hook": None}
        mod.set_axon_ntff_profile_hook = lambda h: state.__setitem__("hook", h)
        mod.get_axon_ntff_profile_hook = lambda: state["hook"]
        sys.modules["antenv.axon_hooks"] = mod
        import antenv
        antenv.axon_hooks = mod
        from trn_agent_boot.trn_boot import _ntff_profile_via_ctypes
        hook = _ntff_profile_via_ctypes("/opt/axon/libaxon_pjrt.so")
        mod.set_axon_ntff_profile_hook(hook)
    except Exception as e:  # profiling is optional; never break the run
        print(f"ntff shim unavailable: {e}", file=sys.stderr)


last_exec_time_ns = None
last_results = None


def kernel(img_embed, cap_embed, lens, Wg1, bg1, Wg2, bg2, Wb1, bb1, Wb2, bb2):
    global last_exec_time_ns, last_results
    img_embed = np.ascontiguousarray(np.asarray(img_embed, dtype=np.float32))
    cap_embed = np.ascontiguousarray(np.asarray(cap_embed, dtype=np.float32))
    lens = np.asarray(lens).astype(np.int64)

    # host-side prep: per-core shards + selection/mask weight matrices
    ident = np.eye(128, dtype=np.float32)
    tt = np.arange(T)
    in_maps = []
    for k in range(NCORES):
        jsl = slice(k * BL, (k + 1) * BL)
        cap_k = cap_embed[jsl].reshape(CAP_ROWS, D)
        img_k = img_embed[jsl].reshape(IMG_ROWS, D)
        lens_k = lens[jsl]
        # wsel[(b,t), b'] = (t < len_b) / len_b if b == b' else 0 ; col 32 = 1
        wsel = np.zeros((BL, T, BL + 1), dtype=np.float32)
        for b in range(BL):
            wsel[b, : lens_k[b], b] = 1.0 / float(lens_k[b])
        wsel[:, :, BL] = 1.0
        simg = np.zeros((BL, R, BL), dtype=np.float32)
        for b in range(BL):
            simg[b, :, b] = 1.0 / R
        in_maps.append({
            "cap": cap_k,
            "img": img_k,
            "wsel": wsel.reshape(CAP_ROWS, BL + 1).astype(BF16_NP),
            "simg": simg.reshape(IMG_ROWS, BL).astype(BF16_NP),
            "wg1": np.ascontiguousarray(Wg1).astype(BF16_NP),
            "wb1": np.ascontiguousarray(Wb1).astype(BF16_NP),
            "wg2": np.ascontiguousarray(Wg2).astype(BF16_NP),
            "wb2": np.ascontiguousarray(Wb2).astype(BF16_NP),
            "bg1": np.asarray(bg1, dtype=np.float32).reshape(H, 1),
            "bb1": np.asarray(bb1, dtype=np.float32).reshape(H, 1),
            "bg2p1": (np.asarray(bg2, dtype=np.float32) + 1.0).reshape(D, 1),
            "bb2": np.asarray(bb2, dtype=np.float32).reshape(D, 1),
            "ident": ident,
        })

    nc = _get_nc()
    trace = bool(int(os.environ.get("BASS_KERNEL_TRACE", "0")))
    if trace:
        _install_ntff_shim()
    res = run_bass_kernel_spmd(nc, in_maps, list(range(NCORES)), trace=trace)
    last_exec_time_ns = res.exec_time_ns
    last_results = res

    sim_ij = np.concatenate([res.results[k]["out"] for k in range(NCORES)], axis=0)
    return np.ascontiguousarray(sim_ij.T)
